# revision 22
# baseline (speedup 1.0000x reference)
"""Fused multi-head attention Bass kernel for Trainium2, 8 NeuronCores.

Problem: nn_MultiHeadAttention (B=2, S=2048, D=1024, H=16, dh=64), returns
(layernorm_out [B,S,D], attn [B,H,S,S]).

Sharding: core c handles batch b=c//4 and heads 4*(c%4)..4*(c%4)+4
(data parallel on B, tensor parallel on heads). QKV projections are
column-parallel, fc is row-parallel; the cross-core fc reduction plus
residual+LayerNorm happen on the host (cheap vs. the 536MB attn output).

On-device math (per core), all matmuls in float32r (TF32-class @ full PE
rate):
  Q^T = W_Qc^T X^T, K^T = W_Kc^T X^T   (computed transposed: [dh*4, S])
  V   = X V_c                           ([S, dh*4], + ones column per head)
  S^T[k,q] = K_h Q_h^T / 8  with -8e9*maskT added via identity-matmul PSUM
             pre-load (exp underflows masked entries to exactly 0)
  P^T = exp(S^T/8)  (no max subtraction needed: |scores/8| < ~6)
  [ctx^T | denom] = [V_h | 1]^T P^T     (ones column gives softmax denom)
  attn^T = P^T * bcast(1/denom)         (PE broadcast + DVE multiply)
  out^T_partial = W_fc_c^T ctxN^T       ([D, S] partial over this core's heads)

Host: attn[b,h] = attnT.transpose, out = LN(sum_partials^T + residual).
"""

import sys

for _p in ("/opt/trn_rl_repo",):
    if _p not in sys.path:
        sys.path.append(_p)

import numpy as np
import ml_dtypes

import concourse.bass as bass
import concourse.mybir as mybir
import concourse.tile as tile
from concourse.bass_utils import run_bass_kernel_spmd
from concourse.masks import make_identity

# ---------------------------------------------------------------- constants
B, S, D, H, DH = 2, 2048, 1024, 16, 64
HC = H // 8 * 2  # heads per core = 4 (8 cores: 2 batches x 4 head-groups)
DC = HC * DH  # 256 cols of the projection per core
LN_EPS = 1e-5
QQ = 512  # q-chunk processed per inner pass
NEG = -8.0e9  # mask additive constant (pre-exp-scale; /8 -> -1e9)

F32 = mybir.dt.float32
F32R = mybir.dt.float32r
BF16 = mybir.dt.bfloat16
FP = mybir.ActivationFunctionType

N_KT = S // 128  # 16 k-tiles
N_QQ = S // QQ  # 4 q-chunks
N_ST = S // 128  # 16 s-tiles
KD = D // 128  # 8 contraction chunks for projections


def _split_multi_waits(nc, max_waits=1):
    """walrus here encodes at most `max_waits` sync-waits per instruction;
    Tile emits more (esp. the kernel-tail drain). Split extras into
    same-engine NoOps placed immediately before."""
    ctr = 0
    for blk in nc.m.functions[0].blocks:
        il = blk.instructions
        new_il = []
        for inst in il:
            si = inst.sync_info
            waits = list(si.on_wait) if si is not None and si.on_wait else []
            if len(waits) > max_waits:
                extra, keep = waits[:-max_waits], waits[-max_waits:]
                for i in range(0, len(extra), max_waits):
                    ctr += 1
                    new_il.append(
                        mybir.InstNoOp(
                            name=f"I-waitsplit-{ctr}",
                            engine=inst.engine,
                            ins=[],
                            outs=[],
                            bass_nofuse=True,
                            sync_info=mybir.SyncInfo(
                                on_wait=list(extra[i : i + max_waits]), on_update=[]
                            ),
                        )
                    )
                inst.sync_info = mybir.SyncInfo(
                    on_wait=keep,
                    on_update=list(si.on_update) if si.on_update else [],
                )
            new_il.append(inst)
        il[:] = new_il
    return ctr


def build(waitfix=True):
    nc = bass.Bass(target_bir_lowering=False, debug=False)

    # DRAM I/O (per core)
    xqt_d = nc.dram_tensor("xqt", [D, S], F32R, kind="ExternalInput")
    xkt_d = nc.dram_tensor("xkt", [D, S], F32R, kind="ExternalInput")
    xvt_d = nc.dram_tensor("xvt", [D, S], F32R, kind="ExternalInput")
    maskt_d = nc.dram_tensor("maskt", [S, S], mybir.dt.uint8, kind="ExternalInput")
    wq_d = nc.dram_tensor("wq", [D, DC], F32R, kind="ExternalInput")
    wk_d = nc.dram_tensor("wk", [D, DC], F32R, kind="ExternalInput")
    wv_d = nc.dram_tensor("wv", [D, DC], F32R, kind="ExternalInput")
    wfc_d = nc.dram_tensor("wfc", [DC, D], F32R, kind="ExternalInput")
    attnt_d = nc.dram_tensor("attnt", [HC, S, S], F32R, kind="ExternalOutput")
    outt_d = nc.dram_tensor("outt", [D, S], F32, kind="ExternalOutput")

    with tile.TileContext(nc) as tc:
        with (
            tc.tile_pool(name="const", bufs=1) as const_pool,
            tc.tile_pool(name="w", bufs=2) as w_pool,
            tc.tile_pool(name="xs", bufs=9) as x_pool,
            tc.tile_pool(name="qkt", bufs=1) as qkt_pool,
            tc.tile_pool(name="v", bufs=1) as v_pool,
            tc.tile_pool(name="mask", bufs=28) as mask_pool,
            tc.tile_pool(name="maskb", bufs=18) as maskb_pool,
            tc.tile_pool(name="small", bufs=4) as small_pool,
            tc.tile_pool(name="ctxn", bufs=1) as ctxn_pool,
            tc.tile_pool(name="stage", bufs=3) as stage_pool,
            tc.tile_pool(name="pp", bufs=6, space="PSUM") as mm_ps,
            tc.tile_pool(name="pc", bufs=1, space="PSUM") as ctx_ps,
            tc.tile_pool(name="pb", bufs=1, space="PSUM") as bc_ps,
        ):
            # ---------------- constants
            # plain +1 identity: the host already bakes NEG into maskt values
            neg_ident = const_pool.tile([128, 128], BF16)
            make_identity(nc, neg_ident[:])
            ones_f32 = const_pool.tile([1, 128], F32)
            nc.vector.memset(ones_f32[:], 1.0)
            ones_row = const_pool.tile([1, 128], F32R)
            with nc.allow_low_precision(reason="f32r is 32-bit"):
                nc.vector.tensor_copy(ones_row[:], ones_f32[:])
            vones_f32 = const_pool.tile([128, HC, 1], F32)
            nc.vector.memset(vones_f32[:], 1.0)
            zero_bias = const_pool.tile([128, 1], F32)
            nc.vector.memset(zero_bias[:], 0.0)

            # ---------------- weights (2 shared slots, phase-ordered:
            # wq+wk -> wv frees wq's slot -> wfc frees wk's)
            def load_w(w_d, name):
                w_sb = w_pool.tile([128, KD, DC], F32R, tag="w", name=name)
                for k in range(KD):
                    nc.sync.dma_start(
                        out=w_sb[:, k, :], in_=w_d[k * 128 : (k + 1) * 128, :]
                    )
                return w_sb

            wq_sb = load_w(wq_d, "wq_sb")
            wk_sb = load_w(wk_d, "wk_sb")

            # ---------------- projections
            # Q^T, K^T: [DC, S] as 2 tiles of [128, S]
            qt_sb = qkt_pool.tile([128, 2, S], F32R, tag="qt")
            kt_sb = qkt_pool.tile([128, 2, S], F32R, tag="kt")
            for w_sb, x_d, o_sb in ((wq_sb, xqt_d, qt_sb), (wk_sb, xkt_d, kt_sb)):
                for dt_i in range(2):
                    psums = [mm_ps.tile([128, QQ], F32, tag="proj", name=f"pj{i}") for i in range(4)]
                    for k in range(KD):
                        xt = x_pool.tile([128, S], F32R, tag="x")
                        nc.sync.dma_start(
                            out=xt[:], in_=x_d[k * 128 : (k + 1) * 128, :]
                        )
                        for qc in range(4):
                            nc.tensor.matmul(
                                psums[qc][:],
                                w_sb[:, k, dt_i * 128 : (dt_i + 1) * 128],
                                xt[:, qc * QQ : (qc + 1) * QQ],
                                start=(k == 0),
                                stop=(k == KD - 1),
                            )
                    for qc in range(4):
                        nc.scalar.activation(
                            o_sb[:, dt_i, qc * QQ : (qc + 1) * QQ],
                            psums[qc][:],
                            FP.Copy,
                        )

            # V: [S, DC] as 16 tiles [128, 4, 65] (ones column per head).
            # st-blocks of 4 so X_v loads are big [128, 512] DMAs.
            wv_sb = load_w(wv_d, "wv_sb")
            v_sb = v_pool.tile([128, N_ST, HC, DH + 1], F32R)
            for stb in range(N_ST // 4):
                psums_v = [
                    mm_ps.tile([128, DC], F32, tag="proj", name=f"pv{j}")
                    for j in range(4)
                ]
                for k in range(KD):
                    xt = x_pool.tile([128, QQ], F32R, tag="x", name="xvt_t")
                    nc.sync.dma_start(
                        out=xt[:],
                        in_=xvt_d[
                            k * 128 : (k + 1) * 128, stb * 512 : (stb + 1) * 512
                        ],
                    )
                    for j in range(4):
                        nc.tensor.matmul(
                            psums_v[j][:],
                            xt[:, j * 128 : (j + 1) * 128],
                            wv_sb[:, k, :],
                            start=(k == 0),
                            stop=(k == KD - 1),
                        )
                for j in range(4):
                    st = stb * 4 + j
                    nc.vector.tensor_copy(
                        v_sb[:, st, :, 0:DH],
                        psums_v[j][:].rearrange("p (h d) -> p h d", h=HC),
                    )
                    with nc.allow_low_precision(reason="f32r is 32-bit"):
                        nc.vector.tensor_copy(
                            v_sb[:, st, :, DH : DH + 1], vones_f32[:]
                        )

            # ---------------- attention, per (q-chunk, head)
            # wfc shares the w-pool slots; loaded once V's DMAs are in flight
            wfc_sb = w_pool.tile([128, 2, D], F32R, tag="w", name="wfc_sb")
            for c in range(2):
                nc.sync.dma_start(
                    out=wfc_sb[:, c, :], in_=wfc_d[c * 128 : (c + 1) * 128, :]
                )

            ctxn_sb = ctxn_pool.tile([128, 2, S], F32R)
            for qq in range(N_QQ):
                mask_tiles = []
                for kt in range(N_KT):
                    mu = mask_pool.tile([128, QQ], mybir.dt.uint8, tag="mask")
                    nc.sync.dma_start(
                        out=mu[:],
                        in_=maskt_d[
                            kt * 128 : (kt + 1) * 128, qq * QQ : (qq + 1) * QQ
                        ],
                    )
                    mt = maskb_pool.tile([128, QQ], BF16, tag="maskb")
                    nc.vector.tensor_scalar_mul(mt[:], mu[:], NEG)
                    mask_tiles.append(mt)
                for h in range(HC):
                    t2, p64 = h // 2, (h % 2) * 64
                    # P^T big-tiles: 4 k-tiles packed per tile, slots shared
                    # with the projection-phase X stream (tag "x")
                    p_big = [
                        x_pool.tile([128, 4, QQ], F32R, tag="x", name=f"pb{g}")
                        for g in range(N_KT // 4)
                    ]
                    # pair-batch the identity(mask) matmuls to halve LDW reloads
                    for ktp in range(0, N_KT, 2):
                        psums = [
                            mm_ps.tile([128, QQ], F32, tag="proj", name=f"sc{j}")
                            for j in range(2)
                        ]
                        for j in range(2):
                            nc.tensor.matmul(
                                psums[j][:],
                                neg_ident[:],
                                mask_tiles[ktp + j][:],
                                start=True,
                                stop=False,
                                skip_group_check=True,
                            )
                        for j in range(2):
                            kt = ktp + j
                            nc.tensor.matmul(
                                psums[j][:],
                                kt_sb[p64 : p64 + 64, t2, kt * 128 : (kt + 1) * 128],
                                qt_sb[p64 : p64 + 64, t2, qq * QQ : (qq + 1) * QQ],
                                start=False,
                                stop=True,
                                skip_group_check=True,
                            )
                        for j in range(2):
                            kt = ktp + j
                            nc.scalar.activation(
                                p_big[kt // 4][:, kt % 4, :],
                                psums[j][:],
                                FP.Exp,
                                bias=zero_bias[:],
                                scale=0.125,
                            )
                    # PV: [V_h|1]^T @ P^T -> [ctx^T ; denom]
                    ctx_psum = ctx_ps.tile([DH + 1, QQ], F32, tag="ctx")
                    for kt in range(N_KT):
                        nc.tensor.matmul(
                            ctx_psum[:],
                            v_sb[:, kt, h, :],
                            p_big[kt // 4][:, kt % 4, :],
                            start=(kt == 0),
                            stop=(kt == N_KT - 1),
                        )
                    recip = small_pool.tile([1, QQ], F32R, tag="recip")
                    with nc.allow_low_precision(reason="f32r is 32-bit"):
                        nc.vector.reciprocal(recip[:], ctx_psum[DH : DH + 1, :])
                    bc_psum = bc_ps.tile([128, QQ], F32, tag="bc")
                    nc.tensor.matmul(
                        bc_psum[:], ones_row[:], recip[:], start=True, stop=True
                    )
                    bc_sb = stage_pool.tile([128, QQ], F32R, tag="bc_sb")
                    nc.vector.tensor_copy(bc_sb[:], bc_psum[:])
                    # normalize ctx rows into ctxn (rows h%2*64..+64)
                    nc.vector.tensor_mul(
                        ctxn_sb[p64 : p64 + 64, t2, qq * QQ : (qq + 1) * QQ],
                        ctx_psum[0:DH, :],
                        bc_sb[0:DH, :],
                    )
                    # normalize P in place and dump
                    for kt in range(N_KT):
                        pslice = p_big[kt // 4][:, kt % 4, :]
                        nc.vector.tensor_mul(pslice, pslice, bc_sb[:])
                        nc.sync.dma_start(
                            out=attnt_d[
                                h,
                                kt * 128 : (kt + 1) * 128,
                                qq * QQ : (qq + 1) * QQ,
                            ],
                            in_=pslice,
                        )

                # fc for this q-chunk (ctxN cols qq*QQ complete for all heads)
                for dt_i in range(8):
                    psum_fc = mm_ps.tile([128, QQ], F32, tag="proj", name="fcp")
                    for c in range(2):
                        nc.tensor.matmul(
                            psum_fc[:],
                            wfc_sb[:, c, dt_i * 128 : (dt_i + 1) * 128],
                            ctxn_sb[:, c, qq * QQ : (qq + 1) * QQ],
                            start=(c == 0),
                            stop=(c == 1),
                        )
                    st_t = stage_pool.tile([128, QQ], F32, tag="fcst")
                    nc.scalar.activation(st_t[:], psum_fc[:], FP.Copy)
                    nc.sync.dma_start(
                        out=outt_d[
                            dt_i * 128 : (dt_i + 1) * 128, qq * QQ : (qq + 1) * QQ
                        ],
                        in_=st_t[:],
                    )

    if waitfix:
        _split_multi_waits(nc, max_waits=1)
    return nc


_NC_CACHE = None


def _get_nc():
    global _NC_CACHE
    if _NC_CACHE is None:
        _NC_CACHE = build()
    return _NC_CACHE


def kernel(input_q, input_k, input_v, attn_mask, W_Q, W_K, W_V, W_fc, _timing=None):
    input_q = np.asarray(input_q, dtype=np.float32)
    input_k = np.asarray(input_k, dtype=np.float32)
    input_v = np.asarray(input_v, dtype=np.float32)
    attn_mask = np.asarray(attn_mask)
    W_Q = np.asarray(W_Q, dtype=np.float32)
    W_K = np.asarray(W_K, dtype=np.float32)
    W_V = np.asarray(W_V, dtype=np.float32)
    W_fc = np.asarray(W_fc, dtype=np.float32)

    nc = _get_nc()

    # per-batch host prep (shared across the 4 cores of each batch)
    xqt = [np.ascontiguousarray(input_q[b].T) for b in range(B)]
    xkt = [np.ascontiguousarray(input_k[b].T) for b in range(B)]
    xvt = [np.ascontiguousarray(input_v[b].T) for b in range(B)]
    maskt = [
        np.ascontiguousarray(attn_mask[b].T).view(np.uint8) for b in range(B)
    ]

    in_maps = []
    for core in range(8):
        b, g = core // 4, core % 4
        cols = slice(g * DC, (g + 1) * DC)
        in_maps.append(
            {
                "xqt": xqt[b],
                "xkt": xkt[b],
                "xvt": xvt[b],
                "maskt": maskt[b],
                "wq": np.ascontiguousarray(W_Q[:, cols]),
                "wk": np.ascontiguousarray(W_K[:, cols]),
                "wv": np.ascontiguousarray(W_V[:, cols]),
                "wfc": np.ascontiguousarray(W_fc[cols, :]),
            }
        )

    res = run_bass_kernel_spmd(nc, in_maps, core_ids=list(range(8)))
    if _timing is not None:
        _timing["exec_time_ns"] = res.exec_time_ns

    # ---------------- gather / host epilogue
    attn = np.empty((B, H, S, S), dtype=np.float32)
    for core in range(8):
        b, g = core // 4, core % 4
        at = res.results[core]["attnt"]  # [HC, S(k), S(q)]
        attn[b, g * HC : (g + 1) * HC] = at.transpose(0, 2, 1)

    out = np.empty((B, S, D), dtype=np.float32)
    for b in range(B):
        acc = np.zeros((D, S), dtype=np.float32)
        for g in range(4):
            acc += res.results[b * 4 + g]["outt"]
        x = acc.T + input_q[b]
        mu = x.mean(axis=-1, keepdims=True, dtype=np.float64)
        var = ((x - mu) ** 2).mean(axis=-1, keepdims=True, dtype=np.float64)
        out[b] = (x - mu) / np.sqrt(var + LN_EPS)

    return (out, attn)
